# revision 27
# baseline (speedup 1.0000x reference)
"""Additive attention (d2l-style) on 8 TRN2 NeuronCores via Bass/Tile.

Problem shapes: B=16, Q=128, K=512, H=128, DQ=DK=DV=128 (all f32).

Sharding: every core runs the SAME graph over ALL 16 batch elements but
only a 16-query slice of each (core c owns q in [16c, 16c+16)). This
keeps SPMD work perfectly uniform while letting the graph be specialized
to the actual valid_lens: per batch b, only FD_b = round_up(vl_b, 2)
key columns are ever touched (the softmax weight of the rest is exactly
0), which cuts the dominant tanh-features work ~2x in expectation.
The graph is rebuilt (and recompiled, a few seconds) per distinct
valid_lens tuple.

Pipeline (per core):
  prologue:
    PE : qpT_all = W_q^T @ [all 16 q-slices]   (one f32 matmul, (H, 256))
    PE : kpT_cat = W_k^T @ [keys of all batches, pruned to FD_b and
         concatenated] in 512-column chunks -> (H, sum FD_b) bf16
  per batch section b (16 queries x FD_b keys):
    DVE: g[:, j*FD:(j+1)*FD] = kpT_b + qpT[:, j]   (bf16 broadcast add)
    ACT: t = tanh(g) -> bf16, one (128, 16*FD_b) tile
    PE : sc[j, :FD_b] += w_v . t_j  (bf16 matvec, j-th column of a
         zero-padded (H,16) stationary tile -> row j of the PSUM tile)
    ACT: expo[:, :vl] = exp(sc[:, :vl]) -> bf16, accum_out = row sums
         (no mask needed: only live keys are computed; no max-subtraction:
         |scores| <= sum|w_v| ~ 10)
    DVE: zero expo tail of the last 128-key tile
    PE : attn^T via transposes; av = attn^T.T @ values  (bf16)
    DVE: out = av * (1/rowsum) -> f32 -> DMA out
"""

import numpy as np
from contextlib import ExitStack

B, Q, K, H, D = 16, 128, 512, 128, 128
NCORES = 8
QC = Q // NCORES  # 16 queries per core per batch
MASK_VALUE = -1000000.0


def _section_order(fds):
    """Processing order of batch sections: smallest first (fast pipeline
    ramp), then the rest by descending size (small sections at the tail)."""
    order = sorted(range(B), key=lambda b: fds[b])
    return [order[0]] + order[:0:-1]


def _build_graph(vls):
    """vls: per-batch valid_lens, baked into the graph."""
    from concourse import bacc, tile, mybir, masks

    f32 = mybir.dt.float32
    bf16 = mybir.dt.bfloat16
    AF = mybir.ActivationFunctionType

    fds = [min(K, ((v + 1) // 2) * 2) for v in vls]
    border = _section_order(fds)
    ofds = [fds[b] for b in border]
    offs = np.concatenate([[0], np.cumsum(ofds)]).astype(int)
    sfd = int(offs[-1])
    n_chunks = (sfd + 511) // 512

    nc = bacc.Bacc("TRN2", target_bir_lowering=False, num_devices=NCORES)

    qT = nc.dram_tensor("qT", [D, B * QC], f32, kind="ExternalInput").ap()
    kT = nc.dram_tensor("kT", [D, sfd], bf16, kind="ExternalInput").ap()
    vals = nc.dram_tensor("vals", [B, K, D], bf16, kind="ExternalInput").ap()
    Wq = nc.dram_tensor("Wq", [D, H], f32, kind="ExternalInput").ap()
    Wk = nc.dram_tensor("Wk", [D, H], bf16, kind="ExternalInput").ap()
    wv = nc.dram_tensor("wv", [H, 1], f32, kind="ExternalInput").ap()
    out = nc.dram_tensor("out", [B, QC, D], f32, kind="ExternalOutput").ap()

    with tile.TileContext(nc) as tc:
        with (
            tc.tile_pool(name="const", bufs=1) as const,
            tc.tile_pool(name="inp", bufs=3) as inp,
            tc.tile_pool(name="g_pool", bufs=4) as g_pool,
            tc.tile_pool(name="t_pool", bufs=4) as t_pool,
            tc.tile_pool(name="soft_sb", bufs=3) as soft_sb,
            tc.tile_pool(name="at_sb", bufs=3) as at_sb,
            tc.tile_pool(name="out_sb", bufs=3) as out_sb,
        ):
            wq_t = const.tile([D, H], f32)
            nc.sync.dma_start(wq_t[:], Wq[:])
            wk_t = const.tile([D, H], bf16)
            nc.sync.dma_start(wk_t[:], Wk[:])
            wv_f32 = const.tile([H, 1], f32)
            nc.sync.dma_start(wv_f32[:], wv[:])
            # wv_diag[:, j*QC+j] = w_v, else 0: matvec with the (H, QC)
            # slice j writes w_v . t into row j of the PSUM section tile
            # and zeros into the other QC-1 rows.
            wv_diag = const.tile([H, QC * QC], bf16)
            nc.vector.memset(wv_diag[:], 0.0)
            for j in range(QC):
                nc.vector.tensor_copy(
                    wv_diag[:, j * QC + j : j * QC + j + 1], wv_f32[:]
                )
            ident = const.tile([QC, QC], bf16)
            masks.make_identity(nc, ident[:])

            # projections: q batched up-front; k-chunks produced just-in-time
            # inside the section loop (engine streams are in-order, so
            # emitting all k-chunk copies first would stall section 0's
            # adds behind the whole prologue)
            kpT = const.tile([H, sfd], bf16)
            kck_pool = ExitStack()
            kck = kck_pool.enter_context(tc.tile_pool(name="kck", bufs=3))
            proj_ps = kck_pool.enter_context(
                tc.tile_pool(name="proj_ps", bufs=2, space="PSUM")
            )
            qT_sb = const.tile([D, B * QC], f32)
            nc.sync.dma_start(qT_sb[:], qT[:])
            qp_ps = proj_ps.tile([H, 512], f32, tag="kp_ps", name="qp_ps")
            nc.tensor.matmul(
                qp_ps[:, : B * QC], wq_t[:], qT_sb[:], start=True, stop=True
            )
            qpT = const.tile([H, B * QC], f32)
            nc.vector.tensor_copy(qpT[:], qp_ps[:, : B * QC])

            next_chunk = [0]

            def emit_chunks_until(need_hi):
                while next_chunk[0] * 512 < need_hi:
                    ch = next_chunk[0]
                    lo = ch * 512
                    hi = min(sfd, lo + 512)
                    kc = kck.tile([D, 512], bf16, tag="kc", name=f"kc{ch}")
                    nc.sync.dma_start(kc[:, : hi - lo], kT[:, lo:hi])
                    kp_ps = proj_ps.tile([H, 512], f32, tag="kp_ps",
                                         name=f"kp_ps{ch}")
                    nc.tensor.matmul(
                        kp_ps[:, : hi - lo], wk_t[:], kc[:, : hi - lo],
                        start=True, stop=True,
                    )
                    nc.vector.tensor_copy(kpT[:, lo:hi], kp_ps[:, : hi - lo])
                    next_chunk[0] += 1

            sec_ps_ctx = ExitStack()
            sc_ps = sec_ps_ctx.enter_context(
                tc.tile_pool(name="sc_ps", bufs=3, space="PSUM")
            )
            at_ps = sec_ps_ctx.enter_context(
                tc.tile_pool(name="at_ps", bufs=1, space="PSUM")
            )
            av_ps = sec_ps_ctx.enter_context(
                tc.tile_pool(name="av_ps", bufs=2, space="PSUM")
            )
            for si in range(B):
                b = border[si]
                vl = int(vls[b])
                fd = fds[b]
                off = int(offs[si])
                nkt = (fd + 127) // 128  # 128-key tiles touched
                emit_chunks_until(off + fd)

                vals_sb = inp.tile([128, nkt * D], bf16, tag="vals_sb")
                for kt in range(nkt):
                    nc.gpsimd.dma_start(
                        vals_sb[:, kt * D : (kt + 1) * D],
                        vals[b, kt * 128 : (kt + 1) * 128, :],
                    )

                g = g_pool.tile([H, QC * fd], bf16, tag="g")
                for j in range(QC):
                    nc.vector.tensor_scalar_add(
                        g[:, j * fd : (j + 1) * fd],
                        kpT[:, off : off + fd],
                        qpT[:, b * QC + j : b * QC + j + 1],
                    )
                tt = t_pool.tile([H, QC * fd], bf16, tag="tt")
                nc.scalar.activation(tt[:], g[:], AF.Tanh)

                sc = sc_ps.tile([QC, fd], f32, tag="sc")
                for j in range(QC):
                    nc.tensor.matmul(
                        sc[:],
                        wv_diag[:, j * QC : (j + 1) * QC],
                        tt[:, j * fd : (j + 1) * fd],
                        start=(j == 0),
                        stop=(j == QC - 1),
                        skip_group_check=True,
                    )

                # softmax over the vl live keys (free axis)
                expo = soft_sb.tile([QC, nkt * 128], bf16, tag="expo")
                sumexp = soft_sb.tile([QC, 1], f32, tag="sumexp")
                if vl < nkt * 128:
                    # zero the dead tail first; exp below rewrites [0, vl)
                    nc.vector.memset(expo[:, (vl // 2) * 2 :], 0.0)
                nc.scalar.activation(
                    expo[:, :vl], sc[:, :vl], AF.Exp, accum_out=sumexp[:]
                )
                rec = soft_sb.tile([QC, 1], f32, tag="rec")
                nc.vector.reciprocal(rec[:], sumexp[:])

                av = av_ps.tile([QC, D], f32, tag="av")
                for kt in range(nkt):
                    aT_ps = at_ps.tile([128, QC], bf16, tag="aT_ps")
                    nc.tensor.transpose(
                        aT_ps[:], expo[:, kt * 128 : (kt + 1) * 128], ident[:]
                    )
                    aT = at_sb.tile([128, QC], bf16, tag="aT")
                    nc.vector.tensor_copy(aT[:], aT_ps[:])
                    nc.tensor.matmul(
                        av[:],
                        aT[:],
                        vals_sb[:, kt * D : (kt + 1) * D],
                        start=(kt == 0),
                        stop=(kt == nkt - 1),
                    )
                ot = out_sb.tile([QC, D], f32, tag="ot")
                nc.vector.tensor_scalar_mul(ot[:], av[:], rec[:])
                nc.gpsimd.dma_start(out[b], ot[:])
            sec_ps_ctx.close()
            kck_pool.close()

    nc.finalize()
    return nc


_NC_CACHE = {}


def _prep(queries, keys, values, valid_lens, W_q, W_k, w_v):
    """Returns (nc, in_maps) for the given full inputs."""
    import ml_dtypes

    bf = ml_dtypes.bfloat16
    queries = np.asarray(queries, dtype=np.float32)
    keys = np.asarray(keys, dtype=np.float32)
    values = np.asarray(values, dtype=np.float32)
    valid_lens = np.asarray(valid_lens).astype(np.int64)
    W_q = np.asarray(W_q, dtype=np.float32)
    W_k = np.asarray(W_k, dtype=np.float32)
    w_v = np.asarray(w_v, dtype=np.float32)

    vls = tuple(int(v) for v in valid_lens)
    if vls not in _NC_CACHE:
        _NC_CACHE[vls] = _build_graph(vls)
    nc = _NC_CACHE[vls]

    fds = [min(K, ((v + 1) // 2) * 2) for v in vls]
    border = _section_order(fds)
    # keys^T pruned to fd_b columns, concatenated in section order
    keysT = keys.transpose(0, 2, 1)  # (B, D, K)
    kT_cat = np.concatenate(
        [keysT[b, :, : fds[b]] for b in border], axis=1
    ).astype(bf)
    vals_bf = values.astype(bf)
    wv2 = np.ascontiguousarray(w_v.reshape(H, 1))
    Wk_bf = W_k.astype(bf)

    in_maps = []
    for c in range(NCORES):
        # (D, B*QC): all batches' q-slices for this core, batch-major
        qT_c = np.ascontiguousarray(
            queries[:, c * QC : (c + 1) * QC, :].transpose(2, 0, 1).reshape(D, B * QC)
        )
        in_maps.append(
            {
                "qT": qT_c,
                "kT": kT_cat,
                "vals": vals_bf,
                "Wq": W_q,
                "Wk": Wk_bf,
                "wv": wv2,
            }
        )
    return nc, in_maps


def _gather(res):
    out = np.empty((B, Q, D), dtype=np.float32)
    for c in range(NCORES):
        out[:, c * QC : (c + 1) * QC, :] = res.results[c]["out"]
    return out


def kernel(queries, keys, values, valid_lens, W_q, W_k, w_v):
    from concourse.bass_utils import run_bass_kernel_spmd

    nc, in_maps = _prep(queries, keys, values, valid_lens, W_q, W_k, w_v)
    res = run_bass_kernel_spmd(nc, in_maps, core_ids=list(range(NCORES)))
    return _gather(res)


# revision 28
# speedup vs baseline: 1.0347x; 1.0347x over previous
"""Additive attention (d2l-style) on 8 TRN2 NeuronCores via Bass/Tile.

Problem shapes: B=16, Q=128, K=512, H=128, DQ=DK=DV=128 (all f32).

Sharding: every core runs the SAME graph over ALL 16 batch elements but
only a 16-query slice of each (core c owns q in [16c, 16c+16)). This
keeps SPMD work perfectly uniform while letting the graph be specialized
to the actual valid_lens: per batch b, only FD_b = round_up(vl_b, 2)
key columns are ever touched (the softmax weight of the rest is exactly
0), which cuts the dominant tanh-features work ~2x in expectation.
The graph is rebuilt (and recompiled, a few seconds) per distinct
valid_lens tuple.

Pipeline (per core):
  prologue:
    PE : qpT_all = W_q^T @ [all 16 q-slices]   (one f32 matmul, (H, 256))
    PE : kpT_cat = W_k^T @ [keys of all batches, pruned to FD_b and
         concatenated] in 512-column chunks -> (H, sum FD_b) bf16
  per batch section b (16 queries x FD_b keys):
    DVE: g[:, j*FD:(j+1)*FD] = kpT_b + qpT[:, j]   (bf16 broadcast add)
    ACT: t = tanh(g) -> bf16, one (128, 16*FD_b) tile
    PE : sc[j, :FD_b] += w_v . t_j  (bf16 matvec, j-th column of a
         zero-padded (H,16) stationary tile -> row j of the PSUM tile)
    ACT: expo[:, :vl] = exp(sc[:, :vl]) -> bf16, accum_out = row sums
         (no mask needed: only live keys are computed; no max-subtraction:
         |scores| <= sum|w_v| ~ 10)
    DVE: zero expo tail of the last 128-key tile
    PE : attn^T via transposes; av = attn^T.T @ values  (bf16)
    DVE: out = av * (1/rowsum) -> f32 -> DMA out
"""

import numpy as np
from contextlib import ExitStack

B, Q, K, H, D = 16, 128, 512, 128, 128
NCORES = 8
QC = Q // NCORES  # 16 queries per core per batch
MASK_VALUE = -1000000.0


def _section_order(fds):
    """Processing order of batch sections: smallest first (fast pipeline
    ramp), then the rest by descending size (small sections at the tail)."""
    order = sorted(range(B), key=lambda b: fds[b])
    return [order[0]] + order[:0:-1]


def _build_graph(vls):
    """vls: per-batch valid_lens, baked into the graph."""
    from concourse import bacc, tile, mybir, masks

    f32 = mybir.dt.float32
    bf16 = mybir.dt.bfloat16
    AF = mybir.ActivationFunctionType

    fds = [min(K, ((v + 1) // 2) * 2) for v in vls]
    border = _section_order(fds)
    ofds = [fds[b] for b in border]
    offs = np.concatenate([[0], np.cumsum(ofds)]).astype(int)
    sfd = int(offs[-1])
    n_chunks = (sfd + 511) // 512

    nc = bacc.Bacc("TRN2", target_bir_lowering=False, num_devices=NCORES)

    qT = nc.dram_tensor("qT", [D, B * QC], f32, kind="ExternalInput").ap()
    kT = nc.dram_tensor("kT", [D, sfd], bf16, kind="ExternalInput").ap()
    vals = nc.dram_tensor("vals", [B, K, D], bf16, kind="ExternalInput").ap()
    Wq = nc.dram_tensor("Wq", [D, H], f32, kind="ExternalInput").ap()
    Wk = nc.dram_tensor("Wk", [D, H], bf16, kind="ExternalInput").ap()
    wv = nc.dram_tensor("wv", [H, 1], f32, kind="ExternalInput").ap()
    out = nc.dram_tensor("out", [B, QC, D], f32, kind="ExternalOutput").ap()

    with tile.TileContext(nc) as tc:
        with (
            tc.tile_pool(name="const", bufs=1) as const,
            tc.tile_pool(name="inp", bufs=3) as inp,
            tc.tile_pool(name="g_pool", bufs=4) as g_pool,
            tc.tile_pool(name="t_pool", bufs=4) as t_pool,
            tc.tile_pool(name="soft_sb", bufs=3) as soft_sb,
            tc.tile_pool(name="at_sb", bufs=3) as at_sb,
            tc.tile_pool(name="out_sb", bufs=3) as out_sb,
        ):
            wq_t = const.tile([D, H], f32)
            nc.sync.dma_start(wq_t[:], Wq[:])
            wk_t = const.tile([D, H], bf16)
            nc.sync.dma_start(wk_t[:], Wk[:])
            wv_f32 = const.tile([H, 1], f32)
            nc.sync.dma_start(wv_f32[:], wv[:])
            # wv_diag[:, j*QC+j] = w_v, else 0: matvec with the (H, QC)
            # slice j writes w_v . t into row j of the PSUM section tile
            # and zeros into the other QC-1 rows.
            wv_diag = const.tile([H, QC * QC], bf16)
            nc.vector.memset(wv_diag[:], 0.0)
            for j in range(QC):
                nc.vector.tensor_copy(
                    wv_diag[:, j * QC + j : j * QC + j + 1], wv_f32[:]
                )
            ident = const.tile([QC, QC], bf16)
            masks.make_identity(nc, ident[:])

            # projections: q batched up-front; k-chunks produced just-in-time
            # inside the section loop (engine streams are in-order, so
            # emitting all k-chunk copies first would stall section 0's
            # adds behind the whole prologue)
            kpT = const.tile([H, sfd], bf16)
            kck_pool = ExitStack()
            kck = kck_pool.enter_context(tc.tile_pool(name="kck", bufs=3))
            proj_ps = kck_pool.enter_context(
                tc.tile_pool(name="proj_ps", bufs=2, space="PSUM")
            )
            qT_sb = const.tile([D, B * QC], f32)
            nc.sync.dma_start(qT_sb[:], qT[:])
            qp_ps = proj_ps.tile([H, 512], f32, tag="kp_ps", name="qp_ps")
            nc.tensor.matmul(
                qp_ps[:, : B * QC], wq_t[:], qT_sb[:], start=True, stop=True
            )
            qpT = const.tile([H, B * QC], f32)
            nc.vector.tensor_copy(qpT[:], qp_ps[:, : B * QC])

            next_chunk = [0]

            def emit_chunks_until(need_hi):
                while next_chunk[0] * 512 < need_hi:
                    ch = next_chunk[0]
                    lo = ch * 512
                    hi = min(sfd, lo + 512)
                    kc = kck.tile([D, 512], bf16, tag="kc", name=f"kc{ch}")
                    nc.sync.dma_start(kc[:, : hi - lo], kT[:, lo:hi])
                    kp_ps = proj_ps.tile([H, 512], f32, tag="kp_ps",
                                         name=f"kp_ps{ch}")
                    nc.tensor.matmul(
                        kp_ps[:, : hi - lo], wk_t[:], kc[:, : hi - lo],
                        start=True, stop=True,
                    )
                    nc.vector.tensor_copy(kpT[:, lo:hi], kp_ps[:, : hi - lo])
                    next_chunk[0] += 1

            sec_ps_ctx = ExitStack()
            sc_ps = sec_ps_ctx.enter_context(
                tc.tile_pool(name="sc_ps", bufs=3, space="PSUM")
            )
            at_ps = sec_ps_ctx.enter_context(
                tc.tile_pool(name="at_ps", bufs=1, space="PSUM")
            )
            av_ps = sec_ps_ctx.enter_context(
                tc.tile_pool(name="av_ps", bufs=2, space="PSUM")
            )
            for si in range(B):
                b = border[si]
                vl = int(vls[b])
                fd = fds[b]
                off = int(offs[si])
                nkt = (fd + 127) // 128  # 128-key tiles touched
                emit_chunks_until(off + fd)

                vals_sb = inp.tile([128, nkt * D], bf16, tag="vals_sb")
                for kt in range(nkt):
                    nc.sync.dma_start(
                        vals_sb[:, kt * D : (kt + 1) * D],
                        vals[b, kt * 128 : (kt + 1) * 128, :],
                    )

                g = g_pool.tile([H, QC * fd], bf16, tag="g")
                for j in range(QC):
                    nc.vector.tensor_scalar_add(
                        g[:, j * fd : (j + 1) * fd],
                        kpT[:, off : off + fd],
                        qpT[:, b * QC + j : b * QC + j + 1],
                    )
                tt = t_pool.tile([H, QC * fd], bf16, tag="tt")
                nc.scalar.activation(tt[:], g[:], AF.Tanh)

                sc = sc_ps.tile([QC, fd], f32, tag="sc")
                for j in range(QC):
                    nc.tensor.matmul(
                        sc[:],
                        wv_diag[:, j * QC : (j + 1) * QC],
                        tt[:, j * fd : (j + 1) * fd],
                        start=(j == 0),
                        stop=(j == QC - 1),
                        skip_group_check=True,
                    )

                # softmax over the vl live keys (free axis)
                expo = soft_sb.tile([QC, nkt * 128], bf16, tag="expo")
                sumexp = soft_sb.tile([QC, 1], f32, tag="sumexp")
                if vl < nkt * 128:
                    # zero the dead tail first; exp below rewrites [0, vl)
                    nc.gpsimd.memset(expo[:, (vl // 2) * 2 :], 0.0)
                nc.scalar.activation(
                    expo[:, :vl], sc[:, :vl], AF.Exp, accum_out=sumexp[:]
                )
                rec = soft_sb.tile([QC, 1], f32, tag="rec")
                nc.vector.reciprocal(rec[:], sumexp[:])

                av = av_ps.tile([QC, D], f32, tag="av")
                for kt in range(nkt):
                    aT_ps = at_ps.tile([128, QC], bf16, tag="aT_ps")
                    nc.tensor.transpose(
                        aT_ps[:], expo[:, kt * 128 : (kt + 1) * 128], ident[:]
                    )
                    aT = at_sb.tile([128, QC], bf16, tag="aT")
                    nc.vector.tensor_copy(aT[:], aT_ps[:])
                    nc.tensor.matmul(
                        av[:],
                        aT[:],
                        vals_sb[:, kt * D : (kt + 1) * D],
                        start=(kt == 0),
                        stop=(kt == nkt - 1),
                    )
                ot = out_sb.tile([QC, D], f32, tag="ot")
                nc.vector.tensor_scalar_mul(ot[:], av[:], rec[:])
                nc.sync.dma_start(out[b], ot[:])
            sec_ps_ctx.close()
            kck_pool.close()

    nc.finalize()
    return nc


_NC_CACHE = {}


def _prep(queries, keys, values, valid_lens, W_q, W_k, w_v):
    """Returns (nc, in_maps) for the given full inputs."""
    import ml_dtypes

    bf = ml_dtypes.bfloat16
    queries = np.asarray(queries, dtype=np.float32)
    keys = np.asarray(keys, dtype=np.float32)
    values = np.asarray(values, dtype=np.float32)
    valid_lens = np.asarray(valid_lens).astype(np.int64)
    W_q = np.asarray(W_q, dtype=np.float32)
    W_k = np.asarray(W_k, dtype=np.float32)
    w_v = np.asarray(w_v, dtype=np.float32)

    vls = tuple(int(v) for v in valid_lens)
    if vls not in _NC_CACHE:
        _NC_CACHE[vls] = _build_graph(vls)
    nc = _NC_CACHE[vls]

    fds = [min(K, ((v + 1) // 2) * 2) for v in vls]
    border = _section_order(fds)
    # keys^T pruned to fd_b columns, concatenated in section order
    keysT = keys.transpose(0, 2, 1)  # (B, D, K)
    kT_cat = np.concatenate(
        [keysT[b, :, : fds[b]] for b in border], axis=1
    ).astype(bf)
    vals_bf = values.astype(bf)
    wv2 = np.ascontiguousarray(w_v.reshape(H, 1))
    Wk_bf = W_k.astype(bf)

    in_maps = []
    for c in range(NCORES):
        # (D, B*QC): all batches' q-slices for this core, batch-major
        qT_c = np.ascontiguousarray(
            queries[:, c * QC : (c + 1) * QC, :].transpose(2, 0, 1).reshape(D, B * QC)
        )
        in_maps.append(
            {
                "qT": qT_c,
                "kT": kT_cat,
                "vals": vals_bf,
                "Wq": W_q,
                "Wk": Wk_bf,
                "wv": wv2,
            }
        )
    return nc, in_maps


def _gather(res):
    out = np.empty((B, Q, D), dtype=np.float32)
    for c in range(NCORES):
        out[:, c * QC : (c + 1) * QC, :] = res.results[c]["out"]
    return out


def kernel(queries, keys, values, valid_lens, W_q, W_k, w_v):
    from concourse.bass_utils import run_bass_kernel_spmd

    nc, in_maps = _prep(queries, keys, values, valid_lens, W_q, W_k, w_v)
    res = run_bass_kernel_spmd(nc, in_maps, core_ids=list(range(NCORES)))
    return _gather(res)
